# revision 18
# baseline (speedup 1.0000x reference)
"""Trainium2 Bass kernel: 16-head attention with LoRA (B=2, N=2048, C=1024).

Sharding: batch x sequence rows across 8 cores (core c: batch c//4, rows
(c%4)*512). Heads stay whole per core; K/V are all-gathered over the 4-core
batch group in chunks interleaved with compute. LoRA is folded into the
weights on the host (W_eff = W + 2*B@A), softmax normalization is deferred
and batched. Everything on device is computed transposed (feature dim on
partitions); the host transposes the per-core [1024, 512] output slabs back.
"""

import os
from contextlib import ExitStack

import numpy as np
import ml_dtypes

import concourse.bass as bass
import concourse.mybir as mybir
import concourse.tile as tile
from concourse.bass_utils import run_bass_kernel_spmd

B, N, C, H, D = 2, 2048, 1024, 16, 64
R = 512          # query rows per core
KT = N // 128    # 16 seq tiles of 128
BF = mybir.dt.bfloat16
F32 = mybir.dt.float32
GROUPS = [[0, 1, 2, 3], [4, 5, 6, 7]]


def _ap(src, dims):
    """Rebuild an AP keeping its partition dim but with custom free dims."""
    return bass.AP(tensor=src.tensor, offset=src.offset,
                   ap=[list(src.ap[0])] + [list(d) for d in dims])


def build():
    nc = bass.Bass()
    xT = nc.declare_dram_parameter("xT", [C, R], BF, isOutput=False)
    wkT = nc.declare_dram_parameter("wkT", [C, C], BF, isOutput=False)
    wqT = nc.declare_dram_parameter("wqT", [C, C], BF, isOutput=False)
    wvT = nc.declare_dram_parameter("wvT", [C, C], BF, isOutput=False)
    projT = nc.declare_dram_parameter("projT", [C, C], BF, isOutput=False)
    biasT = nc.declare_dram_parameter("biasT", [1, C], BF, isOutput=False)
    outT = nc.declare_dram_parameter("outT", [C, R], F32, isOutput=True)

    with tile.TileContext(nc) as tc, ExitStack() as ctx:
        dram = ctx.enter_context(tc.tile_pool(name="dram", bufs=1, space="DRAM"))
        warm_in = dram.tile([1, 128], BF)
        warm_out = dram.tile([4, 128], BF)
        kA_b = dram.tile([4 * 128, R], BF)
        kB_b = dram.tile([4 * 128, R], BF)
        kA_g = dram.tile([4 * 4 * 128, R], BF)
        kB_g = dram.tile([4 * 4 * 128, R], BF)
        vA_b = dram.tile([R, 520], BF)
        vB_b = dram.tile([R, 520], BF)
        vA_g = dram.tile([N, 520], BF)
        vB_g = dram.tile([N, 520], BF)
        den_d = dram.tile([16, R], F32)
        den_rd = dram.tile([16, R], BF)

        cst = ctx.enter_context(tc.tile_pool(name="cst", bufs=1))

        # warm-up collective at t~0: absorbs the ncfw barrier/setup latency.
        # Emitted first so its trigger is the first gpsimd/vector work.
        warm_s = cst.tile([1, 128], BF)
        with tc.high_priority():
            nc.vector.memset(warm_s, 1.0)
            nc.gpsimd.dma_start(out=warm_in, in_=warm_s)
            nc.gpsimd.collective_compute(
                "AllGather", mybir.AluOpType.bypass,
                ins=[warm_in.opt()], outs=[warm_out.opt()],
                replica_groups=GROUPS)

        atn = ctx.enter_context(tc.tile_pool(name="atn", bufs=1))
        xT_s = cst.tile([128, 8, R], BF)
        nc.sync.dma_start(out=xT_s, in_=xT[:, :].rearrange("(kt p) r -> p kt r", p=128))
        # k/q weight slots are recycled for the gathered-V tiles ("big" tag):
        # wk -> buf0, wq -> buf1, then vA reuses buf0 (after P1), vB buf1
        wkT_s = atn.tile([128, 8, C], BF, tag="big", bufs=2, name="wkT_s")
        nc.sync.dma_start(out=wkT_s, in_=wkT[:, :].rearrange("(kt p) c -> p kt c", p=128))
        wvT_s = cst.tile([128, 8, C], BF)
        nc.sync.dma_start(out=wvT_s, in_=wvT[:, :].rearrange("(kt p) c -> p kt c", p=128))
        wqT_s = atn.tile([128, 8, C], BF, tag="big", bufs=2, name="wqT_s")
        nc.sync.dma_start(out=wqT_s, in_=wqT[:, :].rearrange("(kt p) c -> p kt c", p=128))
        projT_s = cst.tile([128, 8, C], BF)
        nc.sync.dma_start(out=projT_s, in_=projT[:, :].rearrange("(kt p) c -> p kt c", p=128))
        biasT_s = cst.tile([1, C], BF)
        nc.sync.dma_start(out=biasT_s, in_=biasT[:, :])

        ones_s = cst.tile([1, R], BF)
        nc.vector.memset(ones_s, 1.0)
        kT_ls = cst.tile([128, 8, R], BF)
        qT_s = cst.tile([128, 8, R], BF)
        v_ls = cst.tile([128, 4, 1040], BF)
        nc.vector.memset(v_ls, 1.0)
        att_un = cst.tile([128, 8, R], BF)
        f6_s = cst.tile([128, 8, R], BF)
        att_s = cst.tile([128, 8, R], BF)
        rb_s = cst.tile([128, 8, R], BF)
        ps = ctx.enter_context(tc.tile_pool(name="ps", bufs=1, space="PSUM"))

        # ---- P1a: k columns 0..511 (heads 0-7), trigger K1 gather
        def k_block(ct):
            k_ps = ps.tile([128, R], F32, tag="mm", bufs=2, name=f"k_{ct}")
            for kt in range(8):
                nc.tensor.matmul(k_ps, wkT_s[:, kt, ct * 128:(ct + 1) * 128],
                                 xT_s[:, kt, :], start=(kt == 0), stop=(kt == 7))
            nc.vector.tensor_copy(kT_ls[:, ct, :], k_ps)

        def v_block(vc, rt):
            v_ps = ps.tile([128, R], F32, tag="mm", bufs=2, name=f"v_{vc}_{rt}")
            for kt in range(8):
                nc.tensor.matmul(v_ps, xT_s[:, kt, rt * 128:(rt + 1) * 128],
                                 wvT_s[:, kt, vc * 512:(vc + 1) * 512],
                                 start=(kt == 0), stop=(kt == 7))
            dst = v_ls[:, rt, vc * 520:(vc + 1) * 520]
            nc.vector.tensor_copy(_ap(dst, [[65, 8], [1, 64]]),
                                  v_ps[:, :].rearrange("p (h e) -> p h e", e=64))

        for ct in range(4):
            k_block(ct)
        with tc.high_priority():
            nc.gpsimd.dma_start(
                out=kA_b[:, :].rearrange("(ct p) r -> p ct r", p=128),
                in_=kT_ls[:, 0:4, :])
            nc.gpsimd.collective_compute(
                "AllGather", mybir.AluOpType.bypass,
                ins=[kA_b.opt()], outs=[kA_g.opt()], replica_groups=GROUPS)

        # ---- P2a: v columns 0..511 (heads 0-7), trigger V1 gather
        for rt in range(4):
            v_block(0, rt)
        with tc.high_priority():
            nc.gpsimd.dma_start(
                out=vA_b[:, :].rearrange("(rt p) c -> p rt c", p=128),
                in_=v_ls[:, :, 0:520])
            nc.gpsimd.collective_compute(
                "AllGather", mybir.AluOpType.bypass,
                ins=[vA_b.opt()], outs=[vA_g.opt()], replica_groups=GROUPS)

        # ---- P1b: k columns 512..1023 (heads 8-15), trigger K2
        for ct in range(4, 8):
            k_block(ct)
        with tc.high_priority():
            nc.gpsimd.dma_start(
                out=kB_b[:, :].rearrange("(ct p) r -> p ct r", p=128),
                in_=kT_ls[:, 4:8, :])
            nc.gpsimd.collective_compute(
                "AllGather", mybir.AluOpType.bypass,
                ins=[kB_b.opt()], outs=[kB_g.opt()], replica_groups=GROUPS)

        # ---- P2b: v columns 512..1023 (heads 8-15), trigger V2
        for rt in range(4):
            v_block(1, rt)
        with tc.high_priority():
            nc.gpsimd.dma_start(
                out=vB_b[:, :].rearrange("(rt p) c -> p rt c", p=128),
                in_=v_ls[:, :, 520:1040])
            nc.gpsimd.collective_compute(
                "AllGather", mybir.AluOpType.bypass,
                ins=[vB_b.opt()], outs=[vB_g.opt()], replica_groups=GROUPS)

        # ---- P3: q
        for ct in range(8):
            q_ps = ps.tile([128, R], F32, tag="mm", bufs=2, name=f"q_{ct}")
            for kt in range(8):
                nc.tensor.matmul(q_ps, wqT_s[:, kt, ct * 128:(ct + 1) * 128],
                                 xT_s[:, kt, :], start=(kt == 0), stop=(kt == 7))
            nc.vector.tensor_copy(qT_s[:, ct, :], q_ps)

        # gathered V -> SBUF in kt chunks so attn@V can start on the first
        # chunk while the rest stages (on gpsimd queue, behind the collectives)
        vA_s = atn.tile([128, KT, 520], BF, tag="big", bufs=2, name="vA_s")
        vB_s = atn.tile([128, KT, 520], BF, tag="big", bufs=2, name="vB_s")
        for vg, vsb in ((vA_g, vA_s), (vB_g, vB_s)):
            for q4 in range(4):
                nc.gpsimd.dma_start(
                    out=vsb[:, q4 * 4:(q4 + 1) * 4, :],
                    in_=vg[q4 * 512:(q4 + 1) * 512, :].rearrange(
                        "(kt p) c -> p kt c", p=128))

        # normalize a contiguous batch of head pairs: one reciprocal for the
        # batch, two partition-broadcast DMAs, one mul per head pair — all on
        # DVE/its DMA queue so nothing blocks the sync-queue kT_p prefetches
        def norm_batch(k0, n):
            den_l = atn.tile([2 * n, R], F32, tag=f"denl{k0}", bufs=1,
                             name=f"denl_{k0}")
            nc.gpsimd.dma_start(out=den_l, in_=den_d[2 * k0:2 * k0 + 2 * n, :])
            den_r = atn.tile([2 * n, R], BF, tag=f"denr{k0}", bufs=1,
                             name=f"denr_{k0}")
            with nc.allow_low_precision(reason="softmax denom recip to bf16"):
                nc.vector.reciprocal(den_r, den_l)
            nc.gpsimd.dma_start(out=den_rd[2 * k0:2 * k0 + 2 * n, :], in_=den_r)
            dr = den_rd[:, :]
            for j in range(2):
                nc.gpsimd.dma_start(
                    out=rb_s[j * 64:(j + 1) * 64, k0:k0 + n, :],
                    in_=bass.AP(tensor=dr.tensor,
                                offset=dr.offset + (2 * k0 + j) * R,
                                ap=[[0, 64], [2 * R, n], [1, R]]))
            for kp in range(k0, k0 + n):
                nc.vector.tensor_mul(att_s[:, kp, :], att_un[:, kp, :],
                                     rb_s[:, kp, :])

        # ---- P4: attention, per head pair
        for kp in range(8):
            kg = (kA_g if kp < 4 else kB_g)[:, :]
            kpo = kp % 4
            vs_ = vA_s if kp < 4 else vB_s
            kT_p = atn.tile([128, 4, R], BF, tag="ktp", bufs=2, name=f"ktp_{kp}")
            nc.sync.dma_start(
                out=kT_p,
                in_=bass.AP(tensor=kg.tensor,
                            offset=kg.offset + kpo * 128 * R,
                            ap=[[R, 128], [4 * 128 * R, 4], [1, R]]))
            ao = [ps.tile([65, R], F32, tag=f"ao{j}", bufs=1, name=f"ao_{kp}_{j}")
                  for j in range(2)]
            # software pipeline: attn@V for tile kt-1 is emitted after the
            # scores+exp for kt, so the PE works while ScalarE runs exp
            def av(kt, ex):
                for j in range(2):
                    hj = 2 * kpo + j
                    nc.tensor.matmul(ao[j], vs_[:, kt, hj * 65:(hj + 1) * 65],
                                     ex[:, j, :],
                                     start=(kt == 0), stop=(kt == KT - 1))
            prev_ex = None
            for kt in range(KT):
                sp = ps.tile([128, 2, R], F32, tag="sp", bufs=2,
                             name=f"sp_{kp}_{kt}")
                for j in range(2):
                    nc.tensor.matmul(
                        sp[:, j, :],
                        kT_p[j * 64:(j + 1) * 64, kt // 4, (kt % 4) * 128:(kt % 4) * 128 + 128],
                        qT_s[j * 64:(j + 1) * 64, kp, :],
                        start=True, stop=True)
                ex = atn.tile([128, 2, R], BF, tag="exps", bufs=16,
                              name=f"ex_{kp}_{kt}")
                nc.scalar.activation(ex, sp, mybir.ActivationFunctionType.Exp,
                                     scale=0.125)
                if kt > 0:
                    av(kt - 1, prev_ex)
                prev_ex = ex
            av(KT - 1, prev_ex)
            # drain denominators + unnormalized numerators
            for j in range(2):
                dstg = atn.tile([65, R], F32, tag="dstg", bufs=2,
                                name=f"dstg_{kp}_{j}")
                nc.vector.tensor_copy(dstg[64:65, :], ao[j][64:65, :])
                nc.gpsimd.dma_start(out=den_d[2 * kp + j:2 * kp + j + 1, :],
                                    in_=dstg[64:65, :])
                if j == 0:
                    nc.vector.tensor_copy(att_un[0:64, kp, :], ao[j][0:64, :])
                else:
                    tmp = atn.tile([64, R], F32, tag="tmpj", bufs=2,
                                   name=f"tmpj_{kp}")
                    nc.vector.tensor_copy(tmp, ao[j][0:64, :])
                    nc.gpsimd.dma_start(out=att_un[64:128, kp, :], in_=tmp)
            if kp == 5:
                norm_batch(0, 6)   # normalize kp 0-5 while kp 6-7 compute
                for ct in range(8):
                    f6_ps = ps.tile([128, R], F32, tag="mm", bufs=2,
                                    name=f"f6_{ct}")
                    for k6 in range(6):
                        nc.tensor.matmul(
                            f6_ps, projT_s[:, k6, ct * 128:(ct + 1) * 128],
                            att_s[:, k6, :], start=(k6 == 0), stop=(k6 == 5))
                    nc.vector.tensor_copy(f6_s[:, ct, :], f6_ps)
        norm_batch(6, 2)

        # ---- P5: remaining projection (kp 6-7) + bias + kp0-5 partial
        for ct in range(8):
            f_ps = ps.tile([128, R], F32, tag="mm", bufs=2, name=f"f_{ct}")
            for kp in (6, 7):
                nc.tensor.matmul(f_ps, projT_s[:, kp, ct * 128:(ct + 1) * 128],
                                 att_s[:, kp, :], start=(kp == 6), stop=False)
            nc.tensor.matmul(f_ps, biasT_s[:, ct * 128:(ct + 1) * 128], ones_s,
                             start=False, stop=True)
            f_s = atn.tile([128, R], F32, tag="fs", bufs=2, name=f"fs_{ct}")
            nc.vector.tensor_add(f_s, f_ps, f6_s[:, ct, :])
            nc.gpsimd.dma_start(out=outT[ct * 128:(ct + 1) * 128, :], in_=f_s)

        # consume the warm-up gather so its DMA completes inside the NEFF
        warm_back = cst.tile([4, 128], BF)
        nc.sync.dma_start(out=warm_back, in_=warm_out[:, :])
    _split_multi_waits(nc)
    return nc


def _split_multi_waits(nc):
    """This container's walrus supports one sync-wait per instruction; move
    extra waits onto preceding same-engine NoOps."""
    n_new = 0
    for bb in nc.m.functions[0].blocks:
        new = []
        for ins in bb.instructions:
            si = getattr(ins, "sync_info", None)
            ow = list(si.on_wait) if si is not None and si.on_wait else []
            if len(ow) > 1:
                for w in ow[:-1]:
                    n_new += 1
                    nop = mybir.InstNoOp(
                        name=f"{ins.name}_sw{n_new}",
                        engine=ins.engine,
                        sync_info=mybir.SyncInfo(on_wait=[w], on_update=[]),
                    )
                    new.append(nop)
                ins.sync_info = mybir.SyncInfo(
                    on_wait=[ow[-1]],
                    on_update=list(si.on_update) if si.on_update else [],
                )
            new.append(ins)
        bb.instructions = new


_NC = None
_LAST = None


def _ensure_ntff_hook():
    """The agent image's antenv lacks axon_hooks; shim it and register the
    ctypes NTFF profiler from trn_boot so trace=True yields exec_time_ns."""
    import sys
    import types
    try:
        import antenv.axon_hooks  # noqa: F401
        return
    except ImportError:
        pass
    mod = types.ModuleType("antenv.axon_hooks")
    holder = [None]
    mod.set_axon_ntff_profile_hook = lambda h: holder.__setitem__(0, h)
    mod.get_axon_ntff_profile_hook = lambda: holder[0]
    sys.modules["antenv.axon_hooks"] = mod
    import antenv
    antenv.axon_hooks = mod
    try:
        sys.path.insert(0, "/root/.axon_site")
        from trn_agent_boot.trn_boot import _ntff_profile_via_ctypes
        mod.set_axon_ntff_profile_hook(
            _ntff_profile_via_ctypes("/opt/axon/libaxon_pjrt.so"))
    except Exception:
        pass


def kernel(**inputs):
    global _NC, _LAST
    bf = ml_dtypes.bfloat16
    x = np.asarray(inputs["x"], np.float32)
    qkv_w = np.asarray(inputs["qkv_w"], np.float32)
    proj_w = np.asarray(inputs["proj_w"], np.float32)
    proj_b = np.asarray(inputs["proj_b"], np.float32)
    a1 = np.asarray(inputs["lora_w1_l1"], np.float32)
    b1 = np.asarray(inputs["lora_w1_l2"], np.float32)
    a2 = np.asarray(inputs["lora_w2_l1"], np.float32)
    b2 = np.asarray(inputs["lora_w2_l2"], np.float32)

    w_eff = qkv_w + 2.0 * (b1 @ a1)
    p_eff = proj_w + 2.0 * (b2 @ a2)
    shared = {
        "wqT": np.ascontiguousarray(w_eff[0:C].T).astype(bf),
        "wkT": np.ascontiguousarray(w_eff[C:2 * C].T).astype(bf),
        "wvT": np.ascontiguousarray(w_eff[2 * C:3 * C].T).astype(bf),
        "projT": np.ascontiguousarray(p_eff.T).astype(bf),
        "biasT": np.ascontiguousarray(proj_b[None, :]).astype(bf),
    }
    in_maps = []
    for c in range(8):
        g, r = divmod(c, 4)
        m = dict(shared)
        m["xT"] = np.ascontiguousarray(x[g, r * R:(r + 1) * R, :].T).astype(bf)
        in_maps.append(m)

    if _NC is None:
        _NC = build()
    trace = os.environ.get("ATT_TRACE", "0") == "1"
    if trace:
        _ensure_ntff_hook()
    _LAST = run_bass_kernel_spmd(_NC, in_maps, core_ids=list(range(8)),
                                 trace=trace)
    out = np.empty((B, N, C), np.float32)
    for c in range(8):
        g, r = divmod(c, 4)
        out[g, r * R:(r + 1) * R, :] = np.asarray(
            _LAST.results[c]["outT"], np.float32).T
    return out
